# revision 2
# baseline (speedup 1.0000x reference)
"""Trainium2 Bass kernel for CorrectedPartialCharges.

out[i] = pc[i] + (total_charge[g] - seg_sum[g]) / n_atoms[g],  g = i // 256

Sharding: graphs are data-parallel across the 8 cores (4096 graphs /
1,048,576 atoms per core); segment sums and the gather-broadcast stay
device-local. On each core, partition p owns 32 contiguous graphs.

Wire format is bf16 (the 2e-2 rel-err budget allows it): node charges are
rounded to bf16 on the host, halving HBM traffic both ways; all device
accumulation is fp32. total_charge is pre-divided by 256 on the host so
the leftover is one fused scalar_tensor_tensor op.

Schedule model: the kernel is HBM-bound (4 MiB/core at ~358 GB/s = ~11.7us
of DMA no matter how loads/stores interleave), so the only scheduling goal
is to keep the HBM busy end-to-end and keep the post-last-load tail short.
  - ALL tile loads go first on the sync HWDGE ring, then ALL stores queue
    behind them on the same ring: loads run at full rate, stores drain the
    accumulated backlog immediately after - no idle gap, no cross-ring
    round-robin halving the load rate.
  - The tiny constants (identity + total-charge) ride the OTHERWISE IDLE
    scalar/ACT HWDGE ring and land before the first tile, so the matmul
    chain is never gated on them (on the SWDGE path they landed ~6us late
    and stalled everything).
  - Tensor: accumulating identity matmuls fold each graph's 256 atoms into
    PSUM_W PSUM columns. Vector: PSUM reduce -> seg, fused leftover, and
    the per-graph adds for its tiles. ACT: per-graph bias adds for the
    late, small tiles (its adds are ~2x slower, so it gets short chains).
  - GpSimd does nothing (SWDGE data was observed to lag HWDGE by ~4us).
"""

import ml_dtypes
import numpy as np

import concourse.bacc as bacc
import concourse.bass as bass  # noqa: F401
import concourse.mybir as mybir
import concourse.tile as tile
from concourse.bass_utils import run_bass_kernel_spmd

N_CORES = 8
ATOMS_PER_GRAPH = 256
N_GRAPHS = 32768
N_ATOMS = N_GRAPHS * ATOMS_PER_GRAPH
P = 128

G_PER_CORE = N_GRAPHS // N_CORES          # 4096 graphs per core
A_PER_CORE = G_PER_CORE * ATOMS_PER_GRAPH  # 1,048,576 atoms per core

# Knobs read by test.py when experimenting.
# Tile widths by index (atoms per partition); tiles are contiguous spans of
# the free dim in index order. Sum must be 8192; each a multiple of 256 with
# width/256 * PSUM_W * 4 <= 2048 (PSUM accumulation bank limit).
TILE_W = (2048, 2048, 2048, 1024, 1024)
PSUM_W = 64               # columns per graph after the matmul pre-reduce
# add engine per tile ("vector" or "scalar"); ACT adds are ~2x slower per
# block, so ACT only owns short (small-width) tiles.
TILE_ADD_ENGINE = {0: "vector", 1: "vector", 2: "vector", 3: "scalar", 4: "scalar"}
# ring for the tile loads / stores ("sync" or "scalar")
LOAD_Q = "sync"
STORE_Q = "sync"
CONST_Q = "scalar"

_TRACE = False
_TRACE_KWARGS = {}


def _build(tile_w=None, add_eng=None, psum_w=None, load_q=None, store_q=None,
           const_q=None):
    tile_w = TILE_W if tile_w is None else tile_w
    add_eng = TILE_ADD_ENGINE if add_eng is None else add_eng
    psum_w = PSUM_W if psum_w is None else psum_w
    load_q = LOAD_Q if load_q is None else load_q
    store_q = STORE_Q if store_q is None else store_q
    const_q = CONST_Q if const_q is None else const_q

    nt = len(tile_w)
    ap_free = A_PER_CORE // P     # 8192 atoms per partition
    gp = G_PER_CORE // P          # 32 graphs per partition
    n_pass = ATOMS_PER_GRAPH // psum_w
    offs = [0]
    for w_ in tile_w:
        offs.append(offs[-1] + w_)
    assert offs[-1] == ap_free
    for w_ in tile_w:
        assert w_ % ATOMS_PER_GRAPH == 0
        assert (w_ // ATOMS_PER_GRAPH) * psum_w * 4 <= 2048, \
            "psum accumulation group must fit one bank"

    nc = bacc.Bacc(None, target_bir_lowering=False, enable_partition_id=False)

    pc = nc.dram_tensor("pc", [A_PER_CORE], mybir.dt.bfloat16, kind="ExternalInput")
    # total_charge / 256, fp32
    tcs = nc.dram_tensor("tcs", [G_PER_CORE], mybir.dt.float32, kind="ExternalInput")
    eye = nc.dram_tensor("eye", [P * P], mybir.dt.bfloat16, kind="ExternalInput")
    out = nc.dram_tensor("out", [A_PER_CORE], mybir.dt.bfloat16, kind="ExternalOutput")

    pc_v = pc[:].rearrange("(p n) -> p n", p=P)
    out_v = out[:].rearrange("(p n) -> p n", p=P)
    tcs_v = tcs[:].rearrange("(p k) -> p k", p=P)
    eye_v = eye[:].rearrange("(p n) -> p n", p=P)

    with tile.TileContext(nc) as tc:
        with (
            tc.tile_pool(name="io", bufs=nt) as io_pool,
            tc.tile_pool(name="small", bufs=2 * nt) as small_pool,
            tc.tile_pool(name="consts", bufs=1) as const_pool,
            tc.tile_pool(name="psum", bufs=min(nt, 6), space="PSUM") as psum_pool,
        ):
            # Constants first, on the (otherwise idle at t=0) ACT ring: they
            # land well before the first tile so compute is never gated.
            ceng = getattr(nc, const_q)
            eye_tile = const_pool.tile([P, P], mybir.dt.bfloat16, tag="eye")
            ceng.dma_start(out=eye_tile[:], in_=eye_v)
            tc_tile = const_pool.tile([P, gp], mybir.dt.float32, tag="tc")
            ceng.dma_start(out=tc_tile[:], in_=tcs_v)

            # Queue every input tile load up front on the load ring; stores
            # are issued later on the same ring and queue behind the loads,
            # which is exactly the drain order we want.
            leng = getattr(nc, load_q)
            xs = []
            for t in range(nt):
                w_ = tile_w[t]
                x = io_pool.tile([P, w_], mybir.dt.bfloat16, tag="x")
                leng.dma_start(out=x[:], in_=pc_v[:, offs[t] : offs[t] + w_])
                xs.append(x)

            seng = getattr(nc, store_q)
            goff = [o // ATOMS_PER_GRAPH for o in offs]  # graph offsets
            for t in range(nt):
                x = xs[t]
                w_ = tile_w[t]
                k = w_ // ATOMS_PER_GRAPH
                x3 = x[:].rearrange("p (k a) -> p k a", a=ATOMS_PER_GRAPH)

                # Fold 256 atoms -> psum_w columns per graph with
                # accumulating identity matmuls on the Tensor engine.
                ps = psum_pool.tile([P, k, psum_w], mybir.dt.float32, tag="ps")
                for s in range(n_pass):
                    nc.tensor.matmul(
                        ps[:],
                        eye_tile[:],
                        x3[:, :, s * psum_w : (s + 1) * psum_w],
                        start=(s == 0),
                        stop=(s == n_pass - 1),
                    )
                seg = small_pool.tile([P, k], mybir.dt.float32, tag="seg")
                nc.vector.reduce_sum(
                    out=seg[:], in_=ps[:], axis=mybir.AxisListType.X
                )

                # left = (seg * -1/256) + tc/256   (fused)
                left = small_pool.tile([P, k], mybir.dt.float32, tag="left")
                nc.vector.scalar_tensor_tensor(
                    out=left[:],
                    in0=seg[:],
                    scalar=-1.0 / ATOMS_PER_GRAPH,
                    in1=tc_tile[:, goff[t] : goff[t] + k],
                    op0=mybir.AluOpType.mult,
                    op1=mybir.AluOpType.add,
                )

                for j in range(k):
                    blk = x[:, j * ATOMS_PER_GRAPH : (j + 1) * ATOMS_PER_GRAPH]
                    if add_eng[t] == "scalar":
                        nc.scalar.add(out=blk, in_=blk, add=left[:, j : j + 1])
                    else:
                        nc.vector.tensor_scalar_add(
                            out=blk, in0=blk, scalar1=left[:, j : j + 1]
                        )
                seng.dma_start(out=out_v[:, offs[t] : offs[t] + w_], in_=x[:])

    nc.finalize()
    return nc


_NC_CACHE = {}


def _get_nc():
    key = (TILE_W, tuple(sorted(TILE_ADD_ENGINE.items())), PSUM_W,
           LOAD_Q, STORE_Q, CONST_Q)
    if key not in _NC_CACHE:
        _NC_CACHE[key] = _build()
    return _NC_CACHE[key]


def _cpu_fallback(pc, total_charge, batch, n_atoms):
    num_segments = n_atoms.shape[0]
    seg = np.bincount(batch, weights=pc.astype(np.float64), minlength=num_segments)
    leftover = (total_charge - seg.astype(np.float32)) / n_atoms.astype(np.float32)
    return (pc + leftover[batch]).astype(np.float32)


_EYE = None


def kernel(**inputs) -> np.ndarray:
    global _EYE
    pc = np.ascontiguousarray(
        np.asarray(inputs["node_outputs"], dtype=np.float32).reshape(-1)
    )
    total_charge = np.ascontiguousarray(
        np.asarray(inputs["total_charge"], dtype=np.float32).reshape(-1)
    )
    batch = np.asarray(inputs["batch"]).reshape(-1)
    n_atoms = np.ascontiguousarray(np.asarray(inputs["n_atoms"], dtype=np.int32).reshape(-1))

    # The device kernel hardcodes the uniform 256-atoms-per-graph layout the
    # reference generator produces; anything else goes through numpy.
    if (
        pc.shape[0] != N_ATOMS
        or total_charge.shape[0] != N_GRAPHS
        or not np.all(n_atoms == ATOMS_PER_GRAPH)
        or not np.array_equal(
            batch.astype(np.int64),
            np.arange(N_ATOMS, dtype=np.int64) // ATOMS_PER_GRAPH,
        )
    ):
        return _cpu_fallback(pc, total_charge, batch, n_atoms)

    pc_b = pc.astype(ml_dtypes.bfloat16)
    tcs = (total_charge * (1.0 / ATOMS_PER_GRAPH)).astype(np.float32)
    if _EYE is None:
        _EYE = np.eye(P, dtype=ml_dtypes.bfloat16).reshape(-1)

    nc = _get_nc()
    in_maps = []
    for c in range(N_CORES):
        in_maps.append(
            {
                "pc": pc_b[c * A_PER_CORE : (c + 1) * A_PER_CORE],
                "tcs": tcs[c * G_PER_CORE : (c + 1) * G_PER_CORE],
                "eye": _EYE,
            }
        )
    res = run_bass_kernel_spmd(
        nc, in_maps, list(range(N_CORES)), trace=_TRACE, **_TRACE_KWARGS
    )
    out = np.concatenate([r["out"] for r in res.results]).astype(np.float32)
    if _TRACE:
        kernel.last_results = res
    return out


# revision 10
# speedup vs baseline: 1.0118x; 1.0118x over previous
"""Trainium2 Bass kernel for CorrectedPartialCharges.

out[i] = pc[i] + (total_charge[g] - seg_sum[g]) / n_atoms[g],  g = i // 256

Sharding: graphs are data-parallel across the 8 cores (4096 graphs /
1,048,576 atoms per core); segment sums and the gather-broadcast stay
device-local. On each core, partition p owns 32 contiguous graphs.

Wire format is bf16 (the 2e-2 rel-err budget allows it): node charges are
rounded to bf16 on the host, halving HBM traffic both ways; all device
accumulation is fp32. total_charge is pre-divided by 256 on the host so
the leftover is one fused scalar_tensor_tensor op.

Schedule model: the kernel is HBM-bound (4 MiB/core at ~358 GB/s = ~11.7us
of DMA no matter how loads/stores interleave), so the only scheduling goal
is to keep the HBM busy end-to-end and keep the post-last-load tail short.
  - ALL tile loads go first on the sync HWDGE ring, then ALL stores queue
    behind them on the same ring: loads run at full rate, stores drain the
    accumulated backlog immediately after - no idle gap, no cross-ring
    round-robin halving the load rate.
  - The tiny constants (identity + total-charge) ride the OTHERWISE IDLE
    scalar/ACT HWDGE ring and land before the first tile, so the matmul
    chain is never gated on them (on the SWDGE path they landed ~6us late
    and stalled everything).
  - Tensor: accumulating identity matmuls fold each graph's 256 atoms into
    PSUM_W PSUM columns. Vector: PSUM reduce -> seg, fused leftover, and
    the per-graph adds for its tiles. ACT: per-graph bias adds for the
    late, small tiles (its adds are ~2x slower, so it gets short chains).
  - GpSimd does nothing (SWDGE data was observed to lag HWDGE by ~4us).
"""

import ml_dtypes
import numpy as np

import concourse.bacc as bacc
import concourse.bass as bass  # noqa: F401
import concourse.masks as masks
import concourse.mybir as mybir
import concourse.tile as tile
from concourse.bass_utils import run_bass_kernel_spmd

N_CORES = 8
ATOMS_PER_GRAPH = 256
N_GRAPHS = 32768
N_ATOMS = N_GRAPHS * ATOMS_PER_GRAPH
P = 128

G_PER_CORE = N_GRAPHS // N_CORES          # 4096 graphs per core
A_PER_CORE = G_PER_CORE * ATOMS_PER_GRAPH  # 1,048,576 atoms per core

# Knobs read by test.py when experimenting.
# Tile widths by index (atoms per partition); tiles are contiguous spans of
# the free dim in index order. Sum must be 8192; each a multiple of 256 with
# width/256 * PSUM_W * 4 <= 2048 (PSUM accumulation bank limit).
TILE_W = (1024, 2048, 2048, 2048, 1024)
PSUM_W = 64               # columns per graph after the matmul pre-reduce
# add engine per tile ("vector" or "scalar"); ACT adds are ~2x slower per
# block, so ACT only owns short (small-width) tiles, early so its chain
# overlaps the DVE chain rather than extending the tail.
TILE_ADD_ENGINE = {0: "scalar", 1: "vector", 2: "vector", 3: "vector", 4: "vector"}
# ring for the tile loads / stores ("sync" or "scalar")
LOAD_Q = "sync"
STORE_Q = "sync"

_TRACE = False
_TRACE_KWARGS = {}


def _build(tile_w=None, add_eng=None, psum_w=None, load_q=None, store_q=None):
    tile_w = TILE_W if tile_w is None else tile_w
    add_eng = TILE_ADD_ENGINE if add_eng is None else add_eng
    psum_w = PSUM_W if psum_w is None else psum_w
    load_q = LOAD_Q if load_q is None else load_q
    store_q = STORE_Q if store_q is None else store_q

    nt = len(tile_w)
    ap_free = A_PER_CORE // P     # 8192 atoms per partition
    gp = G_PER_CORE // P          # 32 graphs per partition
    n_pass = ATOMS_PER_GRAPH // psum_w
    offs = [0]
    for w_ in tile_w:
        offs.append(offs[-1] + w_)
    assert offs[-1] == ap_free
    for w_ in tile_w:
        assert w_ % ATOMS_PER_GRAPH == 0
        assert (w_ // ATOMS_PER_GRAPH) * psum_w * 4 <= 2048, \
            "psum accumulation group must fit one bank"

    nc = bacc.Bacc(None, target_bir_lowering=False, enable_partition_id=False)

    pc = nc.dram_tensor("pc", [A_PER_CORE], mybir.dt.bfloat16, kind="ExternalInput")
    # total_charge / 256, fp32
    tcs = nc.dram_tensor("tcs", [G_PER_CORE], mybir.dt.float32, kind="ExternalInput")
    out = nc.dram_tensor("out", [A_PER_CORE], mybir.dt.bfloat16, kind="ExternalOutput")

    pc_v = pc[:].rearrange("(p n) -> p n", p=P)
    out_v = out[:].rearrange("(p n) -> p n", p=P)
    tcs_v = tcs[:].rearrange("(p k) -> p k", p=P)

    with tile.TileContext(nc) as tc:
        with (
            tc.tile_pool(name="io", bufs=nt) as io_pool,
            tc.tile_pool(name="small", bufs=2 * nt) as small_pool,
            tc.tile_pool(name="consts", bufs=1) as const_pool,
            tc.tile_pool(name="psum", bufs=min(nt, 6), space="PSUM") as psum_pool,
        ):
            # Identity for the matmul fold is built on-chip on the (idle at
            # t=0) DVE: a DMA'd identity has 128 tiny 256B descriptors that
            # round-robin with the load ring's 4KB packets at packet
            # granularity and halve its effective bandwidth.
            eye_tile = const_pool.tile([P, P], mybir.dt.bfloat16, tag="eye")
            masks.make_identity(nc, eye_tile[:])

            # Queue every input tile load up front on the load ring; stores
            # are issued later on the same ring and queue behind the loads,
            # which is exactly the drain order we want. The small
            # total-charge vector rides the same ring right after the first
            # tile (it is only needed once that tile's segment sums exist).
            leng = getattr(nc, load_q)
            xs = []
            tc_tile = const_pool.tile([P, gp], mybir.dt.float32, tag="tc")
            for t in range(nt):
                w_ = tile_w[t]
                x = io_pool.tile([P, w_], mybir.dt.bfloat16, tag="x")
                leng.dma_start(out=x[:], in_=pc_v[:, offs[t] : offs[t] + w_])
                xs.append(x)
                if t == 0:
                    leng.dma_start(out=tc_tile[:], in_=tcs_v)

            seng = getattr(nc, store_q)
            goff = [o // ATOMS_PER_GRAPH for o in offs]  # graph offsets
            for t in range(nt):
                x = xs[t]
                w_ = tile_w[t]
                k = w_ // ATOMS_PER_GRAPH
                x3 = x[:].rearrange("p (k a) -> p k a", a=ATOMS_PER_GRAPH)

                # Fold 256 atoms -> psum_w columns per graph with
                # accumulating identity matmuls on the Tensor engine.
                ps = psum_pool.tile([P, k, psum_w], mybir.dt.float32, tag="ps")
                for s in range(n_pass):
                    nc.tensor.matmul(
                        ps[:],
                        eye_tile[:],
                        x3[:, :, s * psum_w : (s + 1) * psum_w],
                        start=(s == 0),
                        stop=(s == n_pass - 1),
                    )
                seg = small_pool.tile([P, k], mybir.dt.float32, tag="seg")
                nc.vector.reduce_sum(
                    out=seg[:], in_=ps[:], axis=mybir.AxisListType.X
                )

                # left = (seg * -1/256) + tc/256   (fused)
                left = small_pool.tile([P, k], mybir.dt.float32, tag="left")
                nc.vector.scalar_tensor_tensor(
                    out=left[:],
                    in0=seg[:],
                    scalar=-1.0 / ATOMS_PER_GRAPH,
                    in1=tc_tile[:, goff[t] : goff[t] + k],
                    op0=mybir.AluOpType.mult,
                    op1=mybir.AluOpType.add,
                )

                for j in range(k):
                    blk = x[:, j * ATOMS_PER_GRAPH : (j + 1) * ATOMS_PER_GRAPH]
                    if add_eng[t] == "scalar":
                        nc.scalar.add(out=blk, in_=blk, add=left[:, j : j + 1])
                    else:
                        nc.vector.tensor_scalar_add(
                            out=blk, in0=blk, scalar1=left[:, j : j + 1]
                        )
                seng.dma_start(out=out_v[:, offs[t] : offs[t] + w_], in_=x[:])

    nc.finalize()
    return nc


_NC_CACHE = {}


def _get_nc():
    key = (TILE_W, tuple(sorted(TILE_ADD_ENGINE.items())), PSUM_W,
           LOAD_Q, STORE_Q)
    if key not in _NC_CACHE:
        _NC_CACHE[key] = _build()
    return _NC_CACHE[key]


def _cpu_fallback(pc, total_charge, batch, n_atoms):
    num_segments = n_atoms.shape[0]
    seg = np.bincount(batch, weights=pc.astype(np.float64), minlength=num_segments)
    leftover = (total_charge - seg.astype(np.float32)) / n_atoms.astype(np.float32)
    return (pc + leftover[batch]).astype(np.float32)


def kernel(**inputs) -> np.ndarray:
    pc = np.ascontiguousarray(
        np.asarray(inputs["node_outputs"], dtype=np.float32).reshape(-1)
    )
    total_charge = np.ascontiguousarray(
        np.asarray(inputs["total_charge"], dtype=np.float32).reshape(-1)
    )
    batch = np.asarray(inputs["batch"]).reshape(-1)
    n_atoms = np.ascontiguousarray(np.asarray(inputs["n_atoms"], dtype=np.int32).reshape(-1))

    # The device kernel hardcodes the uniform 256-atoms-per-graph layout the
    # reference generator produces; anything else goes through numpy.
    if (
        pc.shape[0] != N_ATOMS
        or total_charge.shape[0] != N_GRAPHS
        or not np.all(n_atoms == ATOMS_PER_GRAPH)
        or not np.array_equal(
            batch.astype(np.int64),
            np.arange(N_ATOMS, dtype=np.int64) // ATOMS_PER_GRAPH,
        )
    ):
        return _cpu_fallback(pc, total_charge, batch, n_atoms)

    pc_b = pc.astype(ml_dtypes.bfloat16)
    tcs = (total_charge * (1.0 / ATOMS_PER_GRAPH)).astype(np.float32)

    nc = _get_nc()
    in_maps = []
    for c in range(N_CORES):
        in_maps.append(
            {
                "pc": pc_b[c * A_PER_CORE : (c + 1) * A_PER_CORE],
                "tcs": tcs[c * G_PER_CORE : (c + 1) * G_PER_CORE],
            }
        )
    res = run_bass_kernel_spmd(
        nc, in_maps, list(range(N_CORES)), trace=_TRACE, **_TRACE_KWARGS
    )
    out = np.concatenate([r["out"] for r in res.results]).astype(np.float32)
    if _TRACE:
        kernel.last_results = res
    return out


# revision 11
# speedup vs baseline: 1.0761x; 1.0636x over previous
"""Trainium2 Bass kernel for CorrectedPartialCharges.

out[i] = pc[i] + (total_charge[g] - seg_sum[g]) / n_atoms[g],  g = i // 256

Sharding: graphs are data-parallel across the 8 cores (4096 graphs /
1,048,576 atoms per core); segment sums and the gather-broadcast stay
device-local. On each core, partition p owns 32 contiguous graphs.

Wire format is bf16 (the 2e-2 rel-err budget allows it): node charges are
rounded to bf16 on the host, halving HBM traffic both ways; all device
accumulation is fp32. total_charge is pre-divided by 256 on the host so
the leftover is one fused scalar_tensor_tensor op.

Schedule model: the kernel is HBM-bound (4 MiB/core at ~360 GB/s of DMA no
matter how loads/stores interleave), so the goals are (a) keep the SDMA
streams saturated end-to-end and (b) keep the post-last-load tail short.
  - One HWDGE ring saturates at ~200 GB/s, so the payload is striped over
    BOTH rings (sync + ACT). Per ring, all loads are queued first and
    stores queue behind them: loads run at full rate, then the store
    backlog drains immediately - no idle gap.
  - The identity for the matmul fold is built on-chip (memset +
    affine_select): a DMA'd identity is 128 tiny descriptors that
    round-robin against the ring's 4KB packets and halve its bandwidth.
    The small total-charge vector rides the sync ring right after the
    first tile.
  - A dummy ACTIVATE is emitted right after the load issues so the ~1.3us
    ACT_TABLE_LOAD happens while the first tiles are still in flight
    instead of lazily right before the first real bias-add.
  - Tensor: accumulating identity matmuls fold each graph's 256 atoms into
    PSUM_W PSUM columns. Vector: PSUM reduce -> seg, fused leftover, and
    most per-graph adds. ACT: per-graph bias adds for the small tiles
    (its adds are ~2.5x slower, so it never owns a 2048-wide tile).
"""

import ml_dtypes
import numpy as np

import concourse.bacc as bacc
import concourse.bass as bass  # noqa: F401
import concourse.masks as masks
import concourse.mybir as mybir
import concourse.tile as tile
from concourse.bass_utils import run_bass_kernel_spmd

N_CORES = 8
ATOMS_PER_GRAPH = 256
N_GRAPHS = 32768
N_ATOMS = N_GRAPHS * ATOMS_PER_GRAPH
P = 128

G_PER_CORE = N_GRAPHS // N_CORES          # 4096 graphs per core
A_PER_CORE = G_PER_CORE * ATOMS_PER_GRAPH  # 1,048,576 atoms per core

# Knobs read by test.py when experimenting.
# Tile widths by index (atoms per partition); tiles are contiguous spans of
# the free dim in index order. Sum must be 8192; each a multiple of 256 with
# width/256 * PSUM_W * 4 <= 2048 (PSUM accumulation bank limit).
TILE_W = (1024, 2048, 2048, 2048, 1024)
PSUM_W = 32               # columns per graph after the matmul pre-reduce
# processing (emission) order of the tiles
ORDER = (0, 1, 2, 4, 3)
# add engine per tile ("vector" | "scalar" | "gpsimd")
TILE_ADD_ENGINE = {0: "scalar", 1: "vector", 2: "vector", 3: "vector", 4: "scalar"}
# HWDGE ring per tile for load / store ("sync" | "scalar")
LOAD_RING = {0: "sync", 1: "scalar", 2: "sync", 3: "scalar", 4: "sync"}
STORE_RING = {0: "sync", 1: "scalar", 2: "sync", 3: "scalar", 4: "sync"}
TC_RING = "sync"          # ring for the total-charge vector (after 1st load)

_TRACE = False
_TRACE_KWARGS = {}


def _build(tile_w=None, order=None, add_eng=None, psum_w=None, load_ring=None,
           store_ring=None, tc_ring=None):
    tile_w = TILE_W if tile_w is None else tile_w
    order = ORDER if order is None else order
    add_eng = TILE_ADD_ENGINE if add_eng is None else add_eng
    psum_w = PSUM_W if psum_w is None else psum_w
    load_ring = LOAD_RING if load_ring is None else load_ring
    store_ring = STORE_RING if store_ring is None else store_ring
    tc_ring = TC_RING if tc_ring is None else tc_ring

    nt = len(tile_w)
    ap_free = A_PER_CORE // P     # 8192 atoms per partition
    gp = G_PER_CORE // P          # 32 graphs per partition
    n_pass = ATOMS_PER_GRAPH // psum_w
    offs = [0]
    for w_ in tile_w:
        offs.append(offs[-1] + w_)
    assert offs[-1] == ap_free
    for w_ in tile_w:
        assert w_ % ATOMS_PER_GRAPH == 0
        assert (w_ // ATOMS_PER_GRAPH) * psum_w * 4 <= 2048, \
            "psum accumulation group must fit one bank"
    assert tuple(sorted(order)) == tuple(range(nt))

    nc = bacc.Bacc(None, target_bir_lowering=False, enable_partition_id=False)

    pc = nc.dram_tensor("pc", [A_PER_CORE], mybir.dt.bfloat16, kind="ExternalInput")
    # total_charge / 256, fp32
    tcs = nc.dram_tensor("tcs", [G_PER_CORE], mybir.dt.float32, kind="ExternalInput")
    out = nc.dram_tensor("out", [A_PER_CORE], mybir.dt.bfloat16, kind="ExternalOutput")

    pc_v = pc[:].rearrange("(p n) -> p n", p=P)
    out_v = out[:].rearrange("(p n) -> p n", p=P)
    tcs_v = tcs[:].rearrange("(p k) -> p k", p=P)

    with tile.TileContext(nc) as tc:
        with (
            tc.tile_pool(name="io", bufs=nt) as io_pool,
            tc.tile_pool(name="small", bufs=2 * nt) as small_pool,
            tc.tile_pool(name="consts", bufs=1) as const_pool,
            tc.tile_pool(name="psum", bufs=min(nt, 6), space="PSUM") as psum_pool,
        ):
            # Identity built on-chip on the (idle at t=0) GpSimd engine.
            eye_tile = const_pool.tile([P, P], mybir.dt.bfloat16, tag="eye")
            masks.make_identity(nc, eye_tile[:])

            # Queue every input tile load up front, striped over both HWDGE
            # rings; stores are issued later on the same rings and queue
            # behind the loads. The total-charge vector rides tc_ring right
            # after that ring's first tile.
            xs = []
            tc_tile = const_pool.tile([P, gp], mybir.dt.float32, tag="tc")
            tc_loaded = False
            for t in range(nt):
                w_ = tile_w[t]
                x = io_pool.tile([P, w_], mybir.dt.bfloat16, tag="x")
                getattr(nc, load_ring[t]).dma_start(
                    out=x[:], in_=pc_v[:, offs[t] : offs[t] + w_]
                )
                xs.append(x)
                if not tc_loaded and load_ring[t] == tc_ring:
                    getattr(nc, tc_ring).dma_start(out=tc_tile[:], in_=tcs_v)
                    tc_loaded = True

            # Dummy ACTIVATE: hoists the ~1.3us ACT table load to overlap
            # the in-flight tile loads instead of gating the first real add.
            dummy = const_pool.tile([P, 1], mybir.dt.bfloat16, tag="dummy")
            nc.scalar.add(out=dummy[:], in_=eye_tile[:, 0:1], add=0.0)

            goff = [o // ATOMS_PER_GRAPH for o in offs]  # graph offsets
            for t in order:
                x = xs[t]
                w_ = tile_w[t]
                k = w_ // ATOMS_PER_GRAPH
                x3 = x[:].rearrange("p (k a) -> p k a", a=ATOMS_PER_GRAPH)

                # Fold 256 atoms -> psum_w columns per graph with
                # accumulating identity matmuls on the Tensor engine.
                ps = psum_pool.tile([P, k, psum_w], mybir.dt.float32, tag="ps")
                for s in range(n_pass):
                    nc.tensor.matmul(
                        ps[:],
                        eye_tile[:],
                        x3[:, :, s * psum_w : (s + 1) * psum_w],
                        start=(s == 0),
                        stop=(s == n_pass - 1),
                    )
                seg = small_pool.tile([P, k], mybir.dt.float32, tag="seg")
                nc.vector.reduce_sum(
                    out=seg[:], in_=ps[:], axis=mybir.AxisListType.X
                )

                # left = (seg * -1/256) + tc/256   (fused)
                left = small_pool.tile([P, k], mybir.dt.float32, tag="left")
                nc.vector.scalar_tensor_tensor(
                    out=left[:],
                    in0=seg[:],
                    scalar=-1.0 / ATOMS_PER_GRAPH,
                    in1=tc_tile[:, goff[t] : goff[t] + k],
                    op0=mybir.AluOpType.mult,
                    op1=mybir.AluOpType.add,
                )

                for j in range(k):
                    blk = x[:, j * ATOMS_PER_GRAPH : (j + 1) * ATOMS_PER_GRAPH]
                    if add_eng[t] == "scalar":
                        nc.scalar.add(out=blk, in_=blk, add=left[:, j : j + 1])
                    elif add_eng[t] == "gpsimd":
                        nc.gpsimd.tensor_scalar_add(
                            out=blk, in0=blk, scalar1=left[:, j : j + 1]
                        )
                    else:
                        nc.vector.tensor_scalar_add(
                            out=blk, in0=blk, scalar1=left[:, j : j + 1]
                        )
                getattr(nc, store_ring[t]).dma_start(
                    out=out_v[:, offs[t] : offs[t] + w_], in_=x[:]
                )

    nc.finalize()
    return nc


_NC_CACHE = {}


def _get_nc():
    key = (TILE_W, ORDER, tuple(sorted(TILE_ADD_ENGINE.items())), PSUM_W,
           tuple(sorted(LOAD_RING.items())), tuple(sorted(STORE_RING.items())),
           TC_RING)
    if key not in _NC_CACHE:
        _NC_CACHE[key] = _build()
    return _NC_CACHE[key]


def _cpu_fallback(pc, total_charge, batch, n_atoms):
    num_segments = n_atoms.shape[0]
    seg = np.bincount(batch, weights=pc.astype(np.float64), minlength=num_segments)
    leftover = (total_charge - seg.astype(np.float32)) / n_atoms.astype(np.float32)
    return (pc + leftover[batch]).astype(np.float32)


def kernel(**inputs) -> np.ndarray:
    pc = np.ascontiguousarray(
        np.asarray(inputs["node_outputs"], dtype=np.float32).reshape(-1)
    )
    total_charge = np.ascontiguousarray(
        np.asarray(inputs["total_charge"], dtype=np.float32).reshape(-1)
    )
    batch = np.asarray(inputs["batch"]).reshape(-1)
    n_atoms = np.ascontiguousarray(np.asarray(inputs["n_atoms"], dtype=np.int32).reshape(-1))

    # The device kernel hardcodes the uniform 256-atoms-per-graph layout the
    # reference generator produces; anything else goes through numpy.
    if (
        pc.shape[0] != N_ATOMS
        or total_charge.shape[0] != N_GRAPHS
        or not np.all(n_atoms == ATOMS_PER_GRAPH)
        or not np.array_equal(
            batch.astype(np.int64),
            np.arange(N_ATOMS, dtype=np.int64) // ATOMS_PER_GRAPH,
        )
    ):
        return _cpu_fallback(pc, total_charge, batch, n_atoms)

    pc_b = pc.astype(ml_dtypes.bfloat16)
    tcs = (total_charge * (1.0 / ATOMS_PER_GRAPH)).astype(np.float32)

    nc = _get_nc()
    in_maps = []
    for c in range(N_CORES):
        in_maps.append(
            {
                "pc": pc_b[c * A_PER_CORE : (c + 1) * A_PER_CORE],
                "tcs": tcs[c * G_PER_CORE : (c + 1) * G_PER_CORE],
            }
        )
    res = run_bass_kernel_spmd(
        nc, in_maps, list(range(N_CORES)), trace=_TRACE, **_TRACE_KWARGS
    )
    out = np.concatenate([r["out"] for r in res.results]).astype(np.float32)
    if _TRACE:
        kernel.last_results = res
    return out
